# revision 29
# baseline (speedup 1.0000x reference)
"""Trainium2 Bass kernel for nn_CapsuleLayer (capsule conv + 3-iter routing).

Reference (per batch image, C=128, H=W=32, K=3, pad=1):
  priors[h,w,t,nc] = sum_c x_pad[c,h+i,w+j] * W[t,c,nc] + b[t,nc]
  o = mean_t priors
  3x: dist = sum_cch (o - p_t)^2 ; cw = rsqrt(dist + 1e-4)
      cw = cw / sum_t cw ; o = sum_t cw_t p_t
  out[nc,h,w] = o

Sharding: data-parallel over batch; 8 cores, one image each; weight/bias
replicated; no collectives.

v4 structure (vs v3's s/n/e2 decomposition):
- dist computed DIRECTLY per iteration: d = o - p on DVE (2x bf16 TT),
  square IN-PLACE on the otherwise-idle ACT engine, then one cch-tree on
  DVE.  This deletes v3's iteration-0 ntile square-tree and the whole
  e2/tm/eh smalls path (the ||o||^2 bookkeeping), and all x2 scalings.
  Direct d^2 also has no cancellation, unlike n - s + e2.
- o (= mean / weighted sum) is drained from PSUM on ACT with the 1/9
  scale folded in, instead of a DVE tensor_scalar from PSUM.
- All 4 position-groups advance in lockstep phases per iteration so ACT
  hops (square, rsqrt) hide behind DVE work on other groups.
- bf16 on-chip, fp32 PSUM matmul accumulation.
"""

import numpy as np

C = 128
H = W = 32
B = 8
KK = 9
NCAPS = 32
CCH = 16
NC = NCAPS * CCH  # 512
NIT = 3
NPOS = H * W
CHUNK = 128
GRP = 2  # position-chunks per group
NGRP = NPOS // (CHUNK * GRP)  # 4 groups
PADW = 34

_cache = {}


def _build(with_bias: bool):
    import concourse.bass as bass
    import concourse.tile as tile
    from concourse import bacc, mybir
    from concourse.masks import make_identity

    f32 = mybir.dt.float32
    bf16 = mybir.dt.bfloat16
    X = mybir.AxisListType.X
    ADD = mybir.AluOpType.add
    AF = mybir.ActivationFunctionType

    nc = bacc.Bacc()
    x_d = nc.dram_tensor("x", [C, H, W], f32, kind="ExternalInput")
    w_d = nc.dram_tensor("w", [KK, C, NC], f32, kind="ExternalInput")
    b_d = nc.dram_tensor("b", [KK, NC], f32, kind="ExternalInput")
    out_d = nc.dram_tensor("out", [NC, NPOS], f32, kind="ExternalOutput")

    with tile.TileContext(nc) as tc:
        with (
            tc.tile_pool(name="singles", bufs=1) as singles,
            tc.tile_pool(name="wload", bufs=3) as wload,
            tc.tile_pool(name="big", bufs=2) as big_pool,
            tc.tile_pool(name="h1p", bufs=2) as h1_pool,
            tc.tile_pool(name="wh", bufs=2) as wh_pool,
            tc.tile_pool(name="small", bufs=2) as small_pool,
            tc.tile_pool(name="gstate", bufs=1) as gstate,
            tc.tile_pool(name="ost", bufs=2) as ost_pool,
            tc.tile_pool(name="pp", bufs=3, space="PSUM") as pp,
            tc.tile_pool(name="tpp", bufs=2, space="PSUM") as tpp,
        ):
            # ---- stage inputs directly from HBM (gpsimd cast-DMAs) ----
            # xs[j][c, r*32+w] = x_pad[c, r, w+j] = x[c, r-1, w+j-1] in-range.
            # W loads go first on the gpsimd queue so tap-0 matmuls can start
            # as soon as the xs top halves land; each xs is row-split so
            # chunks 0-3 only wait for the top half.
            # W loads ride the (otherwise idle until output) sync DMA queue
            # in f32, in parallel with the gpsimd cast-DMAs of x; the DVE
            # permute-copy below does the f32->bf16 cast for free.
            wraws = []
            for t in range(KK):
                wt = wload.tile([C, NC], f32, tag="wraw", name=f"wr{t}")
                nc.sync.dma_start(out=wt[:], in_=w_d[t])
                wraws.append(wt)

            xs = []
            xcol = [(1, 32, 0, 31), (0, 32, 0, 32), (0, 31, 1, 32)]
            for j in range(3):
                xj = singles.tile([C, PADW * W], bf16, tag=f"xs{j}")
                xjv = xj[:].rearrange("p (r w) -> p r w", r=PADW)
                nc.gpsimd.memset(xjv[:, 0], 0.0)
                nc.gpsimd.memset(xjv[:, PADW - 1], 0.0)
                d0, d1, s0, s1 = xcol[j]
                if j == 0:
                    nc.gpsimd.memset(xjv[:, 1 : PADW - 1, 0], 0.0)
                if j == 2:
                    nc.gpsimd.memset(xjv[:, 1 : PADW - 1, W - 1], 0.0)
                nc.gpsimd.dma_start(
                    out=xjv[:, 1:18, d0:d1], in_=x_d[:, 0:17, s0:s1]
                )
                xs.append(xj)
            for j in range(3):
                d0, d1, s0, s1 = xcol[j]
                xjv = xs[j][:].rearrange("p (r w) -> p r w", r=PADW)
                nc.gpsimd.dma_start(
                    out=xjv[:, 18 : H + 1, d0:d1], in_=x_d[:, 17:H, s0:s1]
                )

            # per-tap permute (cap,cch)->(cch,cap) on the idle DVE head so
            # the matmul rhs is contiguous (PE cannot stream a strided rhs
            # at rate -- measured 2.2x matmul slowdown)
            wsb = []
            for t in range(KK):
                wp_t = singles.tile([C, CCH, NCAPS], bf16, tag=f"wsbp{t}")
                nc.vector.tensor_copy(
                    out=wp_t[:],
                    in_=wraws[t][:].rearrange(
                        "p (cap cch) -> p cch cap", cch=CCH
                    ),
                )
                wsb.append(wp_t)

            ident = singles.tile([128, 128], f32)
            make_identity(nc, ident[:])

            eps = singles.tile([128, 1], f32)
            nc.gpsimd.memset(eps, 1e-4)

            if with_bias:
                braw = singles.tile([1, KK, NC], bf16)
                nc.gpsimd.dma_start(out=braw[:], in_=b_d[:].unsqueeze(0))
                bsb = singles.tile([1, KK, CCH, NCAPS], bf16)
                nc.scalar.copy(
                    out=bsb[:],
                    in_=braw[:].rearrange("p t (cap cch) -> p t cch cap", cch=CCH),
                )
                ones = singles.tile([1, CHUNK], bf16)
                nc.gpsimd.memset(ones, 1.0)

            # persistent per-group state
            priors = [
                singles.tile(
                    [128, GRP, KK, CCH, NCAPS], bf16,
                    tag=f"pr{g}", name=f"pr{g}",
                )
                for g in range(NGRP)
            ]
            ov = [
                gstate.tile([128, GRP, NC], bf16, tag=f"ov{g}", name=f"ov{g}")
                for g in range(NGRP)
            ]
            alpha = [
                gstate.tile(
                    [128, GRP, KK, NCAPS], bf16, tag=f"al{g}", name=f"al{g}"
                )
                for g in range(NGRP)
            ]

            # ---- priors + mean per chunk (PE + ACT drains, pipelined) ----
            # Taps are matmul'd in pairs sharing one PSUM tile so each ACT
            # drain moves 2 taps (amortizes the per-op startup cost).
            def emit_chunk(ch):
                g, cc = divmod(ch, GRP)
                for t2 in range(5):
                    ntap = 2 if t2 < 4 else 1
                    ps = pp.tile([128, ntap, NC], f32, tag="pp")
                    for tt in range(ntap):
                        t = 2 * t2 + tt
                        i, j = divmod(t, 3)
                        lhsT = xs[j][
                            :, 128 * ch + 32 * i : 128 * ch + 32 * i + 128
                        ]
                        rhs = wsb[t][:].rearrange("p a b -> p (a b)")
                        if with_bias:
                            nc.tensor.matmul(
                                ps[:, tt], lhsT, rhs, start=True, stop=False
                            )
                            brhs = bsb[:, t].rearrange("p a b -> p (a b)")
                            nc.tensor.matmul(
                                ps[:, tt], ones[:], brhs, start=False, stop=True
                            )
                        else:
                            nc.tensor.matmul(
                                ps[:, tt], lhsT, rhs, start=True, stop=True
                            )
                    psv = ps[:].rearrange("p n (a b) -> p n a b", a=CCH)
                    if ch == 0:
                        # DVE is idle before routing starts: help drain the
                        # first chunk's PSUM so group 0 can begin sooner
                        nc.vector.tensor_copy(
                            out=priors[g][:, cc, 2 * t2 : 2 * t2 + ntap],
                            in_=psv,
                        )
                    else:
                        nc.scalar.copy(
                            out=priors[g][:, cc, 2 * t2 : 2 * t2 + ntap],
                            in_=psv,
                        )

            # ov = mean over taps, as a DVE tap-tree on the drained priors
            # (cheaper than a second set of PE accumulation matmuls, which
            # made iteration 0 PE-bound)
            def emit_mean(g):
                pv = priors[g][:].rearrange("p c t a b -> p c t (a b)")
                mh = wh_pool.tile([128, GRP, 4, NC], bf16, tag="wh")
                nc.vector.tensor_add(mh[:], pv[:, :, 0:4], pv[:, :, 4:8])
                nc.vector.tensor_add(
                    mh[:, :, 0:2], mh[:, :, 0:2], mh[:, :, 2:4]
                )
                nc.vector.tensor_add(mh[:, :, 0], mh[:, :, 0], mh[:, :, 1])
                nc.vector.tensor_add(mh[:, :, 1], mh[:, :, 0], pv[:, :, 8])
                nc.vector.tensor_scalar_mul(ov[g][:], mh[:, :, 1], 1.0 / KK)

            # ---- routing: dist = sum_cch (o - p)^2 per (pos, t, cap) ----
            # DVE subtract (2x bf16), square in-place (split 6-tap/3-tap
            # tiles: in it0 the 3-tap half squares on DVE to offload the
            # drain-saturated ACT; later iterations square both on ACT, and
            # the split lets the tree start as soon as the 6-tap half is
            # done), then DVE cch-tree.
            TSPL = 6

            def emit_sub(g, sq_on_dve):
                dA = big_pool.tile(
                    [128, GRP, TSPL, CCH, NCAPS], bf16, tag="bigA"
                )
                dB = big_pool.tile(
                    [128, GRP, KK - TSPL, CCH, NCAPS], bf16, tag="bigB"
                )
                obv = ov[g][:].rearrange("p c (a b) -> p c a b", a=CCH)
                nc.vector.tensor_sub(
                    dA[:],
                    obv.unsqueeze(2).broadcast_to(
                        (128, GRP, TSPL, CCH, NCAPS)
                    ),
                    priors[g][:, :, 0:TSPL],
                )
                nc.vector.tensor_sub(
                    dB[:],
                    obv.unsqueeze(2).broadcast_to(
                        (128, GRP, KK - TSPL, CCH, NCAPS)
                    ),
                    priors[g][:, :, TSPL:KK],
                )
                nc.scalar.activation(out=dA[:], in_=dA[:], func=AF.Square)
                if sq_on_dve:
                    nc.vector.tensor_mul(dB[:], dB[:], dB[:])
                else:
                    nc.scalar.activation(out=dB[:], in_=dB[:], func=AF.Square)
                return dA, dB

            def emit_tree(g, d):
                dA, dB = d
                h1 = h1_pool.tile([128, GRP, KK, 8, NCAPS], bf16, tag="h1")
                nc.vector.tensor_add(
                    h1[:, :, 0:TSPL], dA[:, :, :, 0:8], dA[:, :, :, 8:16]
                )
                nc.vector.tensor_add(
                    h1[:, :, TSPL:KK], dB[:, :, :, 0:8], dB[:, :, :, 8:16]
                )
                nc.vector.tensor_add(
                    h1[:, :, :, 0:4], h1[:, :, :, 0:4], h1[:, :, :, 4:8]
                )
                nc.vector.tensor_add(
                    h1[:, :, :, 0:2], h1[:, :, :, 0:2], h1[:, :, :, 2:4]
                )
                dist = small_pool.tile(
                    [128, GRP, KK, NCAPS], bf16, tag=f"dist{g}", name=f"di{g}"
                )
                nc.vector.tensor_add(dist[:], h1[:, :, :, 0], h1[:, :, :, 1])
                return dist

            for it in range(NIT):
                last = it == NIT - 1
                d_tiles = [None] * NGRP
                dist_tiles = [None] * NGRP
                if it == 0:
                    # chunks run one group ahead of the subs so the ACT FIFO
                    # keeps draining PSUM while a square waits on its DVE sub
                    for g in range(NGRP):
                        emit_chunk(GRP * g)
                        emit_chunk(GRP * g + 1)
                        if g >= 1:
                            emit_mean(g - 1)
                            d_tiles[g - 1] = emit_sub(g - 1, sq_on_dve=True)
                        if g >= 2:
                            dist_tiles[g - 2] = emit_tree(g - 2, d_tiles[g - 2])
                    emit_mean(NGRP - 1)
                    d_tiles[NGRP - 1] = emit_sub(NGRP - 1, sq_on_dve=True)
                    dist_tiles[NGRP - 2] = emit_tree(
                        NGRP - 2, d_tiles[NGRP - 2]
                    )
                else:
                    for g in range(NGRP):
                        d_tiles[g] = emit_sub(g, sq_on_dve=False)
                        if g >= 1:
                            dist_tiles[g - 1] = emit_tree(g - 1, d_tiles[g - 1])
                dist_tiles[NGRP - 1] = emit_tree(NGRP - 1, d_tiles[NGRP - 1])

                # cwu = rsqrt(dist + 1e-4) on ACT (overlaps DVE trees)
                cwu_tiles = []
                for g in range(NGRP):
                    cwu = small_pool.tile(
                        [128, GRP, KK, NCAPS], bf16, tag="cwu"
                    )
                    nc.scalar.activation(
                        out=cwu[:], in_=dist_tiles[g][:],
                        func=AF.Abs_reciprocal_sqrt, bias=eps[:],
                    )
                    cwu_tiles.append(cwu)

                # alpha = cwu / sum_t cwu
                for g in range(NGRP):
                    cwu = cwu_tiles[g]
                    cwsum = small_pool.tile([128, GRP, NCAPS], f32, tag="cwsum")
                    ch_ = small_pool.tile([128, GRP, 4, NCAPS], bf16, tag="eh")
                    nc.vector.tensor_add(ch_[:], cwu[:, :, 0:4], cwu[:, :, 4:8])
                    nc.vector.tensor_add(
                        ch_[:, :, 0:2], ch_[:, :, 0:2], ch_[:, :, 2:4]
                    )
                    nc.vector.tensor_add(ch_[:, :, 0], ch_[:, :, 0], ch_[:, :, 1])
                    nc.vector.tensor_add(cwsum[:], ch_[:, :, 0], cwu[:, :, 8])
                    rs = small_pool.tile([128, GRP, NCAPS], f32, tag="rs")
                    nc.vector.reciprocal_approx_fast(rs[:], cwsum[:])
                    rsb = small_pool.tile([128, GRP, NCAPS], bf16, tag="rsb")
                    nc.vector.tensor_copy(out=rsb[:], in_=rs[:])
                    nc.vector.tensor_mul(
                        alpha[g][:],
                        cwu[:],
                        rsb[:].unsqueeze(2).broadcast_to((128, GRP, KK, NCAPS)),
                    )

                # phase C/D: o' = sum_t alpha_t p_t
                if not last:
                    for g in range(NGRP):
                        wpA = big_pool.tile(
                            [128, GRP, TSPL, CCH, NCAPS], bf16, tag="bigA"
                        )
                        wpB = big_pool.tile(
                            [128, GRP, KK - TSPL, CCH, NCAPS], bf16, tag="bigB"
                        )
                        # per sub-chunk: a tap-sliced alpha broadcast view
                        # exceeds the 3-free-dim ISA AP limit if GRP stays in
                        for cc in range(GRP):
                            abv = alpha[g][:, cc].unsqueeze(2)
                            nc.vector.tensor_mul(
                                wpA[:, cc],
                                priors[g][:, cc, 0:TSPL],
                                abv[:, 0:TSPL].broadcast_to(
                                    (128, TSPL, CCH, NCAPS)
                                ),
                            )
                            nc.vector.tensor_mul(
                                wpB[:, cc],
                                priors[g][:, cc, TSPL:KK],
                                abv[:, TSPL:KK].broadcast_to(
                                    (128, KK - TSPL, CCH, NCAPS)
                                ),
                            )
                        wa = wpA[:].rearrange("p c t a b -> p c t (a b)")
                        wb = wpB[:].rearrange("p c t a b -> p c t (a b)")
                        wh = wh_pool.tile([128, GRP, 4, NC], bf16, tag="wh")
                        nc.vector.tensor_add(
                            wh[:, :, 0:2], wa[:, :, 0:2], wa[:, :, 4:6]
                        )
                        nc.vector.tensor_add(
                            wh[:, :, 2:4], wa[:, :, 2:4], wb[:, :, 0:2]
                        )
                        nc.vector.tensor_add(
                            wh[:, :, 0:2], wh[:, :, 0:2], wh[:, :, 2:4]
                        )
                        nc.vector.tensor_add(wh[:, :, 0], wh[:, :, 0], wh[:, :, 1])
                        nc.vector.tensor_add(ov[g][:], wh[:, :, 0], wb[:, :, 2])
                else:
                    # last iteration: per sub-chunk; output transposes overlap
                    # the other sub-chunks' weighted sums
                    for g in range(NGRP):
                        for cc in range(GRP):
                            ch = GRP * g + cc
                            wpA = big_pool.tile(
                                [128, TSPL, CCH, NCAPS], bf16, tag="bigA"
                            )
                            wpB = big_pool.tile(
                                [128, KK - TSPL, CCH, NCAPS], bf16, tag="bigB"
                            )
                            abv = alpha[g][:, cc].unsqueeze(2)
                            nc.vector.tensor_mul(
                                wpA[:],
                                priors[g][:, cc, 0:TSPL],
                                abv[:, 0:TSPL].broadcast_to(
                                    (128, TSPL, CCH, NCAPS)
                                ),
                            )
                            nc.vector.tensor_mul(
                                wpB[:],
                                priors[g][:, cc, TSPL:KK],
                                abv[:, TSPL:KK].broadcast_to(
                                    (128, KK - TSPL, CCH, NCAPS)
                                ),
                            )
                            wa = wpA[:].rearrange("p t a b -> p t (a b)")
                            wb = wpB[:].rearrange("p t a b -> p t (a b)")
                            wh = wh_pool.tile([128, 4, NC], bf16, tag="wh")
                            nc.vector.tensor_add(
                                wh[:, 0:2], wa[:, 0:2], wa[:, 4:6]
                            )
                            nc.vector.tensor_add(
                                wh[:, 2:4], wa[:, 2:4], wb[:, 0:2]
                            )
                            nc.vector.tensor_add(
                                wh[:, 0:2], wh[:, 0:2], wh[:, 2:4]
                            )
                            nc.vector.tensor_add(wh[:, 0], wh[:, 0], wh[:, 1])
                            onat = ost_pool.tile([128, NC], f32, tag="onat")
                            nc.vector.tensor_add(
                                onat[:].rearrange(
                                    "p (cap cch) -> p cch cap", cch=CCH
                                ),
                                wh[:, 0].rearrange(
                                    "p (cch cap) -> p cch cap", cch=CCH
                                ),
                                wb[:, 2].rearrange(
                                    "p (cch cap) -> p cch cap", cch=CCH
                                ),
                            )
                            ot = ost_pool.tile([128, 4, 128], f32, tag="ostage")
                            for blk in range(4):
                                tp = tpp.tile([128, 128], f32)
                                nc.tensor.transpose(
                                    tp[:],
                                    onat[:, 128 * blk : 128 * (blk + 1)],
                                    ident[:],
                                )
                                nc.scalar.copy(out=ot[:, blk], in_=tp[:])
                            nc.sync.dma_start(
                                out=out_d[
                                    :, 128 * ch : 128 * (ch + 1)
                                ].rearrange("(blk n) q -> n blk q", blk=4),
                                in_=ot[:],
                            )
    nc.compile()
    return nc


def _get_nc(with_bias: bool):
    key = ("nc", with_bias)
    if key not in _cache:
        _cache[key] = _build(with_bias)
    return _cache[key]


def kernel(input, weight, bias, _trace=False):
    from concourse.bass_utils import run_bass_kernel_spmd

    input = np.ascontiguousarray(np.asarray(input, dtype=np.float32))
    w = np.ascontiguousarray(
        np.asarray(weight, dtype=np.float32).reshape(KK, C, NC)
    )
    b = np.ascontiguousarray(np.asarray(bias, dtype=np.float32).reshape(KK, NC))
    with_bias = bool(np.any(b))

    nc = _get_nc(with_bias)
    in_maps = [
        {"x": np.ascontiguousarray(input[i]), "w": w, "b": b} for i in range(B)
    ]
    res = run_bass_kernel_spmd(
        nc, in_maps, core_ids=list(range(B)), trace=_trace
    )
    _cache["last_result"] = res
    out = np.stack(
        [r["out"].reshape(NC, H, W) for r in res.results], axis=0
    )
    return out


# revision 32
# speedup vs baseline: 1.0180x; 1.0180x over previous
"""Trainium2 Bass kernel for nn_CapsuleLayer (capsule conv + 3-iter routing).

Reference (per batch image, C=128, H=W=32, K=3, pad=1):
  priors[h,w,t,nc] = sum_c x_pad[c,h+i,w+j] * W[t,c,nc] + b[t,nc]
  o = mean_t priors
  3x: dist = sum_cch (o - p_t)^2 ; cw = rsqrt(dist + 1e-4)
      cw = cw / sum_t cw ; o = sum_t cw_t p_t
  out[nc,h,w] = o

Sharding: data-parallel over batch; 8 cores, one image each; weight/bias
replicated; no collectives.

v4 structure (vs v3's s/n/e2 decomposition):
- dist computed DIRECTLY per iteration: d = o - p on DVE (2x bf16 TT),
  square IN-PLACE on the otherwise-idle ACT engine, then one cch-tree on
  DVE.  This deletes v3's iteration-0 ntile square-tree and the whole
  e2/tm/eh smalls path (the ||o||^2 bookkeeping), and all x2 scalings.
  Direct d^2 also has no cancellation, unlike n - s + e2.
- o (= mean / weighted sum) is drained from PSUM on ACT with the 1/9
  scale folded in, instead of a DVE tensor_scalar from PSUM.
- All 4 position-groups advance in lockstep phases per iteration so ACT
  hops (square, rsqrt) hide behind DVE work on other groups.
- bf16 on-chip, fp32 PSUM matmul accumulation.
"""

import numpy as np

C = 128
H = W = 32
B = 8
KK = 9
NCAPS = 32
CCH = 16
NC = NCAPS * CCH  # 512
NIT = 3
NPOS = H * W
CHUNK = 128
GRP = 2  # position-chunks per group
NGRP = NPOS // (CHUNK * GRP)  # 4 groups
PADW = 34

_cache = {}


def _build(with_bias: bool):
    import concourse.bass as bass
    import concourse.tile as tile
    from concourse import bacc, mybir
    from concourse.masks import make_identity

    f32 = mybir.dt.float32
    bf16 = mybir.dt.bfloat16
    X = mybir.AxisListType.X
    ADD = mybir.AluOpType.add
    AF = mybir.ActivationFunctionType

    nc = bacc.Bacc()
    x_d = nc.dram_tensor("x", [C, H, W], f32, kind="ExternalInput")
    w_d = nc.dram_tensor("w", [KK, C, NC], f32, kind="ExternalInput")
    b_d = nc.dram_tensor("b", [KK, NC], f32, kind="ExternalInput")
    out_d = nc.dram_tensor("out", [NC, NPOS], f32, kind="ExternalOutput")

    with tile.TileContext(nc) as tc:
        with (
            tc.tile_pool(name="singles", bufs=1) as singles,
            tc.tile_pool(name="wload", bufs=3) as wload,
            tc.tile_pool(name="big", bufs=2) as big_pool,
            tc.tile_pool(name="h1p", bufs=2) as h1_pool,
            tc.tile_pool(name="wh", bufs=2) as wh_pool,
            tc.tile_pool(name="small", bufs=2) as small_pool,
            tc.tile_pool(name="gstate", bufs=1) as gstate,
            tc.tile_pool(name="ost", bufs=2) as ost_pool,
            tc.tile_pool(name="pp", bufs=2, space="PSUM") as pp,
            tc.tile_pool(name="mp", bufs=1, space="PSUM") as mp,
            tc.tile_pool(name="tpp", bufs=2, space="PSUM") as tpp,
        ):
            # ---- stage inputs directly from HBM (gpsimd cast-DMAs) ----
            # xs[j][c, r*32+w] = x_pad[c, r, w+j] = x[c, r-1, w+j-1] in-range.
            # W loads go first on the gpsimd queue so tap-0 matmuls can start
            # as soon as the xs top halves land; each xs is row-split so
            # chunks 0-3 only wait for the top half.
            # W loads ride the (otherwise idle until output) sync DMA queue
            # in f32, in parallel with the gpsimd cast-DMAs of x; the DVE
            # permute-copy below does the f32->bf16 cast for free.
            wraws = []
            for t in range(KK):
                wt = wload.tile([C, NC], f32, tag="wraw", name=f"wr{t}")
                nc.sync.dma_start(out=wt[:], in_=w_d[t])
                wraws.append(wt)

            xs = []
            xcol = [(1, 32, 0, 31), (0, 32, 0, 32), (0, 31, 1, 32)]
            for j in range(3):
                xj = singles.tile([C, PADW * W], bf16, tag=f"xs{j}")
                xjv = xj[:].rearrange("p (r w) -> p r w", r=PADW)
                nc.gpsimd.memset(xjv[:, 0], 0.0)
                nc.gpsimd.memset(xjv[:, PADW - 1], 0.0)
                d0, d1, s0, s1 = xcol[j]
                if j == 0:
                    nc.gpsimd.memset(xjv[:, 1 : PADW - 1, 0], 0.0)
                if j == 2:
                    nc.gpsimd.memset(xjv[:, 1 : PADW - 1, W - 1], 0.0)
                nc.gpsimd.dma_start(
                    out=xjv[:, 1:18, d0:d1], in_=x_d[:, 0:17, s0:s1]
                )
                xs.append(xj)
            for j in range(3):
                d0, d1, s0, s1 = xcol[j]
                xjv = xs[j][:].rearrange("p (r w) -> p r w", r=PADW)
                nc.gpsimd.dma_start(
                    out=xjv[:, 18 : H + 1, d0:d1], in_=x_d[:, 17:H, s0:s1]
                )

            # per-tap permute (cap,cch)->(cch,cap) on the idle DVE head so
            # the matmul rhs is contiguous (PE cannot stream a strided rhs
            # at rate -- measured 2.2x matmul slowdown)
            wsb = []
            for t in range(KK):
                wp_t = singles.tile([C, CCH, NCAPS], bf16, tag=f"wsbp{t}")
                nc.vector.tensor_copy(
                    out=wp_t[:],
                    in_=wraws[t][:].rearrange(
                        "p (cap cch) -> p cch cap", cch=CCH
                    ),
                )
                wsb.append(wp_t)

            ident = singles.tile([128, 128], f32)
            make_identity(nc, ident[:])

            eps = singles.tile([128, 1], f32)
            nc.gpsimd.memset(eps, 1e-4)

            if with_bias:
                braw = singles.tile([1, KK, NC], bf16)
                nc.gpsimd.dma_start(out=braw[:], in_=b_d[:].unsqueeze(0))
                bsb = singles.tile([1, KK, CCH, NCAPS], bf16)
                nc.scalar.copy(
                    out=bsb[:],
                    in_=braw[:].rearrange("p t (cap cch) -> p t cch cap", cch=CCH),
                )
                ones = singles.tile([1, CHUNK], bf16)
                nc.gpsimd.memset(ones, 1.0)

            # persistent per-group state
            priors = [
                singles.tile(
                    [128, GRP, KK, CCH, NCAPS], bf16,
                    tag=f"pr{g}", name=f"pr{g}",
                )
                for g in range(NGRP)
            ]
            ov = [
                gstate.tile([128, GRP, NC], bf16, tag=f"ov{g}", name=f"ov{g}")
                for g in range(NGRP)
            ]
            alpha = [
                gstate.tile(
                    [128, GRP, KK, NCAPS], bf16, tag=f"al{g}", name=f"al{g}"
                )
                for g in range(NGRP)
            ]

            # ---- priors + mean per chunk (PE + ACT drains, pipelined) ----
            # Taps are matmul'd in pairs sharing one PSUM tile so each ACT
            # drain moves 2 taps (amortizes the per-op startup cost).
            def emit_chunk(ch):
                g, cc = divmod(ch, GRP)
                om = mp.tile([128, NC], f32)
                for t2 in range(5):
                    ntap = 2 if t2 < 4 else 1
                    ps = pp.tile([128, ntap, NC], f32, tag="pp")
                    for tt in range(ntap):
                        t = 2 * t2 + tt
                        i, j = divmod(t, 3)
                        lhsT = xs[j][
                            :, 128 * ch + 32 * i : 128 * ch + 32 * i + 128
                        ]
                        rhs = wsb[t][:].rearrange("p a b -> p (a b)")
                        if with_bias:
                            nc.tensor.matmul(
                                ps[:, tt], lhsT, rhs, start=True, stop=False
                            )
                            brhs = bsb[:, t].rearrange("p a b -> p (a b)")
                            nc.tensor.matmul(
                                ps[:, tt], ones[:], brhs, start=False, stop=True
                            )
                        else:
                            nc.tensor.matmul(
                                ps[:, tt], lhsT, rhs, start=True, stop=True
                            )
                        nc.tensor.matmul(
                            om[:], lhsT, rhs, start=(t == 0), stop=(t == KK - 1)
                        )
                        if with_bias:
                            nc.tensor.matmul(
                                om[:], ones[:], brhs, start=False, stop=False,
                                skip_group_check=True,
                            )
                    psv = ps[:].rearrange("p n (a b) -> p n a b", a=CCH)
                    if ch == 0:
                        # DVE is idle before routing starts: help drain the
                        # first chunk's PSUM so group 0 can begin sooner
                        nc.vector.tensor_copy(
                            out=priors[g][:, cc, 2 * t2 : 2 * t2 + ntap],
                            in_=psv,
                        )
                    else:
                        nc.scalar.copy(
                            out=priors[g][:, cc, 2 * t2 : 2 * t2 + ntap],
                            in_=psv,
                        )
                # ov = mean = (1/9) sum_t priors, drained+scaled on ACT so
                # DVE never touches PSUM here
                nc.scalar.activation(
                    out=ov[g][:, cc], in_=om[:], func=AF.Identity,
                    scale=1.0 / KK,
                )

            # ---- routing: dist = sum_cch (o - p)^2 per (pos, t, cap) ----
            # DVE subtract (2x bf16), square in-place (split 6-tap/3-tap
            # tiles: in it0 the 3-tap half squares on DVE to offload the
            # drain-saturated ACT; later iterations square both on ACT, and
            # the split lets the tree start as soon as the 6-tap half is
            # done), then DVE cch-tree.
            TSPL = 6

            def emit_sub(g, sq_on_dve):
                dA = big_pool.tile(
                    [128, GRP, TSPL, CCH, NCAPS], bf16, tag="bigA"
                )
                dB = big_pool.tile(
                    [128, GRP, KK - TSPL, CCH, NCAPS], bf16, tag="bigB"
                )
                obv = ov[g][:].rearrange("p c (a b) -> p c a b", a=CCH)
                nc.vector.tensor_sub(
                    dA[:],
                    obv.unsqueeze(2).broadcast_to(
                        (128, GRP, TSPL, CCH, NCAPS)
                    ),
                    priors[g][:, :, 0:TSPL],
                )
                nc.vector.tensor_sub(
                    dB[:],
                    obv.unsqueeze(2).broadcast_to(
                        (128, GRP, KK - TSPL, CCH, NCAPS)
                    ),
                    priors[g][:, :, TSPL:KK],
                )
                nc.scalar.activation(out=dA[:], in_=dA[:], func=AF.Square)
                if sq_on_dve:
                    nc.vector.tensor_mul(dB[:], dB[:], dB[:])
                else:
                    nc.scalar.activation(out=dB[:], in_=dB[:], func=AF.Square)
                return dA, dB

            def emit_tree(g, d):
                dA, dB = d
                h1 = h1_pool.tile([128, GRP, KK, 8, NCAPS], bf16, tag="h1")
                nc.vector.tensor_add(
                    h1[:, :, 0:TSPL], dA[:, :, :, 0:8], dA[:, :, :, 8:16]
                )
                nc.vector.tensor_add(
                    h1[:, :, TSPL:KK], dB[:, :, :, 0:8], dB[:, :, :, 8:16]
                )
                nc.vector.tensor_add(
                    h1[:, :, :, 0:4], h1[:, :, :, 0:4], h1[:, :, :, 4:8]
                )
                nc.vector.tensor_add(
                    h1[:, :, :, 0:2], h1[:, :, :, 0:2], h1[:, :, :, 2:4]
                )
                dist = small_pool.tile(
                    [128, GRP, KK, NCAPS], bf16, tag=f"dist{g}", name=f"di{g}"
                )
                nc.vector.tensor_add(dist[:], h1[:, :, :, 0], h1[:, :, :, 1])
                return dist

            for it in range(NIT):
                last = it == NIT - 1
                d_tiles = [None] * NGRP
                dist_tiles = [None] * NGRP
                if it == 0:
                    # chunks run one group ahead of the subs so the ACT FIFO
                    # keeps draining PSUM while a square waits on its DVE sub
                    for g in range(NGRP):
                        emit_chunk(GRP * g)
                        emit_chunk(GRP * g + 1)
                        if g >= 1:
                            d_tiles[g - 1] = emit_sub(g - 1, sq_on_dve=True)
                        if g >= 2:
                            dist_tiles[g - 2] = emit_tree(g - 2, d_tiles[g - 2])
                    d_tiles[NGRP - 1] = emit_sub(NGRP - 1, sq_on_dve=True)
                    dist_tiles[NGRP - 2] = emit_tree(
                        NGRP - 2, d_tiles[NGRP - 2]
                    )
                else:
                    for g in range(NGRP):
                        d_tiles[g] = emit_sub(g, sq_on_dve=False)
                        if g >= 1:
                            dist_tiles[g - 1] = emit_tree(g - 1, d_tiles[g - 1])
                dist_tiles[NGRP - 1] = emit_tree(NGRP - 1, d_tiles[NGRP - 1])

                # cwu = rsqrt(dist + 1e-4) on ACT (overlaps DVE trees)
                cwu_tiles = []
                for g in range(NGRP):
                    cwu = small_pool.tile(
                        [128, GRP, KK, NCAPS], bf16, tag="cwu"
                    )
                    nc.scalar.activation(
                        out=cwu[:], in_=dist_tiles[g][:],
                        func=AF.Abs_reciprocal_sqrt, bias=eps[:],
                    )
                    cwu_tiles.append(cwu)

                # alpha = cwu / sum_t cwu
                for g in range(NGRP):
                    cwu = cwu_tiles[g]
                    cwsum = small_pool.tile([128, GRP, NCAPS], f32, tag="cwsum")
                    ch_ = small_pool.tile([128, GRP, 4, NCAPS], bf16, tag="eh")
                    nc.vector.tensor_add(ch_[:], cwu[:, :, 0:4], cwu[:, :, 4:8])
                    nc.vector.tensor_add(
                        ch_[:, :, 0:2], ch_[:, :, 0:2], ch_[:, :, 2:4]
                    )
                    nc.vector.tensor_add(ch_[:, :, 0], ch_[:, :, 0], ch_[:, :, 1])
                    nc.vector.tensor_add(cwsum[:], ch_[:, :, 0], cwu[:, :, 8])
                    rs = small_pool.tile([128, GRP, NCAPS], f32, tag="rs")
                    nc.vector.reciprocal_approx_fast(rs[:], cwsum[:])
                    rsb = small_pool.tile([128, GRP, NCAPS], bf16, tag="rsb")
                    nc.vector.tensor_copy(out=rsb[:], in_=rs[:])
                    nc.vector.tensor_mul(
                        alpha[g][:],
                        cwu[:],
                        rsb[:].unsqueeze(2).broadcast_to((128, GRP, KK, NCAPS)),
                    )

                # phase C/D: o' = sum_t alpha_t p_t
                if not last:
                    for g in range(NGRP):
                        wpA = big_pool.tile(
                            [128, GRP, TSPL, CCH, NCAPS], bf16, tag="bigA"
                        )
                        wpB = big_pool.tile(
                            [128, GRP, KK - TSPL, CCH, NCAPS], bf16, tag="bigB"
                        )
                        # per sub-chunk: a tap-sliced alpha broadcast view
                        # exceeds the 3-free-dim ISA AP limit if GRP stays in
                        for cc in range(GRP):
                            abv = alpha[g][:, cc].unsqueeze(2)
                            nc.vector.tensor_mul(
                                wpA[:, cc],
                                priors[g][:, cc, 0:TSPL],
                                abv[:, 0:TSPL].broadcast_to(
                                    (128, TSPL, CCH, NCAPS)
                                ),
                            )
                            nc.vector.tensor_mul(
                                wpB[:, cc],
                                priors[g][:, cc, TSPL:KK],
                                abv[:, TSPL:KK].broadcast_to(
                                    (128, KK - TSPL, CCH, NCAPS)
                                ),
                            )
                        wa = wpA[:].rearrange("p c t a b -> p c t (a b)")
                        wb = wpB[:].rearrange("p c t a b -> p c t (a b)")
                        wh = wh_pool.tile([128, GRP, 4, NC], bf16, tag="wh")
                        nc.vector.tensor_add(
                            wh[:, :, 0:2], wa[:, :, 0:2], wa[:, :, 4:6]
                        )
                        nc.vector.tensor_add(
                            wh[:, :, 2:4], wa[:, :, 2:4], wb[:, :, 0:2]
                        )
                        nc.vector.tensor_add(
                            wh[:, :, 0:2], wh[:, :, 0:2], wh[:, :, 2:4]
                        )
                        nc.vector.tensor_add(wh[:, :, 0], wh[:, :, 0], wh[:, :, 1])
                        nc.vector.tensor_add(ov[g][:], wh[:, :, 0], wb[:, :, 2])
                else:
                    # last iteration: per sub-chunk; output transposes overlap
                    # the other sub-chunks' weighted sums
                    for g in range(NGRP):
                        for cc in range(GRP):
                            ch = GRP * g + cc
                            wpA = big_pool.tile(
                                [128, TSPL, CCH, NCAPS], bf16, tag="bigA"
                            )
                            wpB = big_pool.tile(
                                [128, KK - TSPL, CCH, NCAPS], bf16, tag="bigB"
                            )
                            abv = alpha[g][:, cc].unsqueeze(2)
                            nc.vector.tensor_mul(
                                wpA[:],
                                priors[g][:, cc, 0:TSPL],
                                abv[:, 0:TSPL].broadcast_to(
                                    (128, TSPL, CCH, NCAPS)
                                ),
                            )
                            nc.vector.tensor_mul(
                                wpB[:],
                                priors[g][:, cc, TSPL:KK],
                                abv[:, TSPL:KK].broadcast_to(
                                    (128, KK - TSPL, CCH, NCAPS)
                                ),
                            )
                            wa = wpA[:].rearrange("p t a b -> p t (a b)")
                            wb = wpB[:].rearrange("p t a b -> p t (a b)")
                            wh = wh_pool.tile([128, 4, NC], bf16, tag="wh")
                            nc.vector.tensor_add(
                                wh[:, 0:2], wa[:, 0:2], wa[:, 4:6]
                            )
                            nc.vector.tensor_add(
                                wh[:, 2:4], wa[:, 2:4], wb[:, 0:2]
                            )
                            nc.vector.tensor_add(
                                wh[:, 0:2], wh[:, 0:2], wh[:, 2:4]
                            )
                            nc.vector.tensor_add(wh[:, 0], wh[:, 0], wh[:, 1])
                            onat = ost_pool.tile([128, NC], f32, tag="onat")
                            nc.vector.tensor_add(
                                onat[:].rearrange(
                                    "p (cap cch) -> p cch cap", cch=CCH
                                ),
                                wh[:, 0].rearrange(
                                    "p (cch cap) -> p cch cap", cch=CCH
                                ),
                                wb[:, 2].rearrange(
                                    "p (cch cap) -> p cch cap", cch=CCH
                                ),
                            )
                            ot = ost_pool.tile([128, 4, 128], f32, tag="ostage")
                            for blk in range(4):
                                tp = tpp.tile([128, 128], f32)
                                nc.tensor.transpose(
                                    tp[:],
                                    onat[:, 128 * blk : 128 * (blk + 1)],
                                    ident[:],
                                )
                                nc.scalar.copy(out=ot[:, blk], in_=tp[:])
                            nc.sync.dma_start(
                                out=out_d[
                                    :, 128 * ch : 128 * (ch + 1)
                                ].rearrange("(blk n) q -> n blk q", blk=4),
                                in_=ot[:],
                            )
    nc.compile()
    return nc


def _get_nc(with_bias: bool):
    key = ("nc", with_bias)
    if key not in _cache:
        _cache[key] = _build(with_bias)
    return _cache[key]


def kernel(input, weight, bias, _trace=False):
    from concourse.bass_utils import run_bass_kernel_spmd

    input = np.ascontiguousarray(np.asarray(input, dtype=np.float32))
    w = np.ascontiguousarray(
        np.asarray(weight, dtype=np.float32).reshape(KK, C, NC)
    )
    b = np.ascontiguousarray(np.asarray(bias, dtype=np.float32).reshape(KK, NC))
    with_bias = bool(np.any(b))

    nc = _get_nc(with_bias)
    in_maps = [
        {"x": np.ascontiguousarray(input[i]), "w": w, "b": b} for i in range(B)
    ]
    res = run_bass_kernel_spmd(
        nc, in_maps, core_ids=list(range(B)), trace=_trace
    )
    _cache["last_result"] = res
    out = np.stack(
        [r["out"].reshape(NC, H, W) for r in res.results], axis=0
    )
    return out
